# revision 1
# baseline (speedup 1.0000x reference)
"""Trainium2 Bass kernel for the AR-LSTM problem (nn_ARLSTM_67473936220828).

Strategy
--------
Pure data-parallel over batch: B=1024 -> 128 batch rows per NeuronCore, no
collectives. The LSTM recurrence (365 hindcast + 48 autoregressive forecast
steps) runs in "gate-major" layout: gate units on SBUF/PSUM partitions and
batch on the free dimension, so the cell elementwise math produces h already
transposed for the next step's matmul (no per-step transpose).

Host-side weight folding: the dynamics embedding is linear, so it collapses
into the LSTM input weights:
    gates_x = x_t @ (W_emb @ W_E^T) + x_s @ W_S^T + (b_emb @ W_E^T + b_ih + b_hh)
giving one augmented stationary operand of K=49 (hindcast) / K=50 (forecast,
extra row = folded autoregressive-feedback vector v_q multiplying Q).

All matmul operands are fp16 (full TensorE rate like bf16, but 10-bit
mantissa = tf32-class precision; fp32 PSUM accumulation); gates/sigmoid/tanh
and the cell state c stay fp32; h is stored fp16. Gate rows are permuted to [g, f, i, o] so tanh(g) - the longest
dependency in the cell update - can start as soon as its PSUM chunks land.
"""

import os
import sys

import numpy as np

for _p in ("/opt/trn_rl_repo", "/root/.axon_site/_ro/trn_rl_repo"):
    if os.path.isdir(_p) and _p not in sys.path:
        sys.path.insert(0, _p)

import ml_dtypes  # noqa: E402

import concourse.bass as bass  # noqa: E402
import concourse.mybir as mybir  # noqa: E402
import concourse.tile as tile  # noqa: E402
from concourse import bacc  # noqa: E402
from concourse.bass_utils import run_bass_kernel_spmd  # noqa: E402

F16 = np.float16
F32 = mybir.dt.float32
HF = mybir.dt.float16

B, T_HC, T_FC, D, E, S, H = 1024, 365, 48, 16, 64, 32, 256
NCORES = 8
BC = B // NCORES          # 128 batch rows per core
G4 = 4 * H                # 1024 gate units
NCH = G4 // 128           # 8 gate chunks of 128 partitions
KH = H // 128             # 2 contraction tiles for the hidden matmul
KX = D + 1                # 17 rows: [Q | x_t] (Q row zero-weighted in hindcast)
NXB = 6                   # x_aug buffer ring depth

Sig = mybir.ActivationFunctionType.Sigmoid
Tanh = mybir.ActivationFunctionType.Tanh
Copy = mybir.ActivationFunctionType.Copy


def _build(T_HC: int = T_HC, T_FC: int = T_FC, repeat: int = 1):
    nc = bacc.Bacc("TRN2")

    # ---- DRAM parameters (per-core shapes) ----
    xhc = nc.declare_dram_parameter("xhc", [T_HC, 2, D, BC], HF, isOutput=False)
    xfc = nc.declare_dram_parameter("xfc", [T_FC, 2, D, BC], HF, isOutput=False)
    wxhc = nc.declare_dram_parameter("wxhc", [64 + KX, G4], HF, isOutput=False)
    wxfc = nc.declare_dram_parameter("wxfc", [64 + KX, G4], HF, isOutput=False)
    # static gate contribution (x_s @ W_S^T + biases), exact via fp16 hi+lo
    shc = nc.declare_dram_parameter("shc", [2, 128, G4], HF, isOutput=False)
    sfc = nc.declare_dram_parameter("sfc", [2, 128, G4], HF, isOutput=False)
    iden = nc.declare_dram_parameter("iden", [128, 128], HF, isOutput=False)
    whh = nc.declare_dram_parameter("whh", [2, 128, KH * G4], HF, isOutput=False)
    wout = nc.declare_dram_parameter("wout", [128, KH], HF, isOutput=False)
    yout = nc.declare_dram_parameter("yout", [T_FC, BC], F32, isOutput=True)

    with tile.TileContext(nc) as tc:
        # ---- persistent tiles (one pool, distinct tags -> distinct slots) ----
        from contextlib import ExitStack

        pools = ExitStack()
        persist = pools.enter_context(tc.tile_pool(name="persist", bufs=1))

        def ptile(shape, dtype, name):
            return persist.tile(shape, dtype, name=name, tag=name)

        wxhc_sb = ptile([64 + KX, G4], HF, "wxhc_sb")
        wxfc_sb = ptile([64 + KX, G4], HF, "wxfc_sb")
        shc_sb = [ptile([128, G4], HF, f"shc{i}") for i in range(2)]
        sfc_sb = [ptile([128, G4], HF, f"sfc{i}") for i in range(2)]
        iden_sb = ptile([128, 128], HF, "iden_sb")
        whh_sb = [ptile([128, KH * G4], HF, f"whh_sb{i}") for i in range(2)]
        wout_sb = ptile([128, KH], HF, "wout_sb")
        cT = ptile([128, H], F32, "cT")
        hT = [ptile([128, H], HF, f"hT{i}") for i in range(2)]
        y_sb = ptile([1, T_FC * BC], F32, "y_sb")
        xaug = [ptile([64 + KX, BC], HF, f"xaug{i}") for i in range(NXB)]

        nc.sync.dma_start(out=wxhc_sb, in_=wxhc[:])
        nc.sync.dma_start(out=wxfc_sb, in_=wxfc[:])
        for i in range(2):
            nc.sync.dma_start(out=whh_sb[i], in_=whh[i])
        nc.sync.dma_start(out=wout_sb, in_=wout[:])
        nc.sync.dma_start(out=iden_sb, in_=iden[:])
        for i in range(2):
            nc.sync.dma_start(out=shc_sb[i], in_=shc[i])
            nc.sync.dma_start(out=sfc_sb[i], in_=sfc[i])
        for i in range(NXB):
            # zero whole buffer once: Q rows, pad region (x rows overwritten
            # by per-step DMAs; Q rows by emit_q at partitions 0/64)
            nc.vector.memset(xaug[i][:, :], 0.0)
        nc.vector.memset(cT, 0.0)
        nc.vector.memset(hT[0], 0.0)

        # ---- rotating pools ----
        # PSUM layout per step: [128, 2048] fp32 = 4 banks, one bank per gate
        # in order [g, f, i, o]. Gate G's two 128-col chunks live at columns
        # G*512 + [0:256]; bank group constraint (start=True clears the whole
        # 2KB bank) forces per-chunk sequential accumulation groups within a
        # bank. Q reuses the g-bank's spare half (cols 384:512) after the
        # gate reads are done.
        GW = 512
        ps = pools.enter_context(tc.tile_pool(name="gps", bufs=2, space="PSUM"))
        acts = pools.enter_context(tc.tile_pool(name="acts", bufs=2))

        def lstm_step(t, wx_sb, s_sb, h_in, h_out):
            """One LSTM cell update. Returns the g-bank psum tile (for Q)."""
            xb = xaug[t % NXB]
            # one single-bank psum tile per gate -> per-gate dependency
            # granularity (activation fires as soon as its own bank is done)
            banks = [
                ps.tile([128, GW], F32, name=f"ps_{n}", tag=f"ps_{n}")
                for n in ("g", "f", "i", "o")
            ]

            def xmm(G, j, start=False):
                # K-stacked compensation in one matmul: lhsT rows
                # [Whi; Whi; 0-pad; Wlo] vs rhs rows [Q|x_hi; 0|x_lo; pad; Q|x_hi]
                nc.tensor.matmul(
                    banks[G][:, j * 128 : (j + 1) * 128],
                    wx_sb[:, (2 * G + j) * 128 : (2 * G + j + 1) * 128],
                    xb[:, :],
                    start=start,
                    stop=False,
                )

            def hmm(G, j, w, k, stop=False):
                nc.tensor.matmul(
                    banks[G][:, j * 128 : (j + 1) * 128],
                    whh_sb[w][
                        :,
                        k * G4 + (2 * G + j) * 128 : k * G4 + (2 * G + j + 1) * 128,
                    ],
                    h_in[:, k * 128 : (k + 1) * 128],
                    start=False,
                    stop=stop,
                )

            def smm(G, part):
                # exact static injection: identity matmul adds S_hi / S_lo
                # for both chunks of the bank in one 256-wide matmul
                nc.tensor.matmul(
                    banks[G][:, 0:256],
                    iden_sb,
                    s_sb[part][:, 2 * G * 128 : (2 * G + 2) * 128],
                    start=False,
                    stop=False,
                )

            # One accumulation group per bank: the opening matmul's start=True
            # clears the whole 2KB bank, so later matmuls join the same group
            # (start=False, first write to untouched columns replaces).
            # x and static matmuls are independent of h and run ahead.
            # x product compensated to ~2^-22: Whi@[Q|x_hi] + Whi@[0|x_lo]
            # + Wlo@[Q|x_hi].
            for G in range(4):
                xmm(G, 0, start=True)
                xmm(G, 1)
            for G in range(4):
                smm(G, 0)
                smm(G, 1)
            tg = acts.tile([128, H], F32, name="tg", tag="tg")
            sf = acts.tile([128, H], F32, name="sf", tag="sf")
            si = acts.tile([128, H], F32, name="si", tag="si")
            so = acts.tile([128, H], HF, name="so", tag="so")
            # interleave: emit each gate's activation right after its bank's
            # matmuls so the semaphore assigner signals per bank
            gate_out = [(tg, Tanh), (sf, Sig), (si, Sig), (so, Sig)]
            for G in range(4):
                for w in range(2):
                    hmm(G, 0, w, 0)
                    hmm(G, 0, w, 1)
                    hmm(G, 1, w, 0)
                    hmm(G, 1, w, 1, stop=(w == 1))
                out, func = gate_out[G]
                nc.scalar.activation(out, banks[G][:, 0:H], func)
            t1 = acts.tile([128, H], F32, name="t1", tag="t1")
            t2 = acts.tile([128, H], F32, name="t2", tag="t2")
            nc.vector.tensor_mul(t1, sf, cT)
            nc.vector.tensor_mul(t2, si, tg)
            nc.vector.tensor_add(cT, t1, t2)
            tc_b = acts.tile([128, H], HF, name="tc_b", tag="tc_b")
            nc.scalar.activation(tc_b, cT, Tanh)
            nc.vector.tensor_mul(h_out, so, tc_b)
            return banks[0]

        def emit_q(g_ps, h_now, t_next, y_row):
            """Q' = h @ W_out (b_out folded on host: y adds it back, feedback
            absorbs it via the bias row). Write y row / next feedback row."""
            q_ps = g_ps[0:1, 384:512]
            for k in range(KH):
                nc.tensor.matmul(
                    q_ps,
                    wout_sb[:, k : k + 1],
                    h_now[:, k * 128 : (k + 1) * 128],
                    start=(k == 0),
                    stop=(k == KH - 1),
                )
            if y_row is not None:
                nc.scalar.activation(
                    y_sb[0:1, y_row * BC : (y_row + 1) * BC], q_ps, Copy
                )
            if t_next is not None:
                nb = xaug[t_next % NXB]
                nc.scalar.activation(nb[0:1, :], q_ps, Copy)
                nc.scalar.activation(nb[64:65, :], q_ps, Copy)

        # ---- hindcast ----  (repeat>1 is a timing-benchmark mode)
        g_ps = None
        for t in range(repeat * T_HC):
            xbuf = xaug[t % NXB]
            nc.sync.dma_start(out=xbuf[1 : 1 + D, :], in_=xhc[t % T_HC, 0])
            nc.sync.dma_start(out=xbuf[KX + 1 : KX + 1 + D, :], in_=xhc[t % T_HC, 1])
            nc.sync.dma_start(out=xbuf[65 : 65 + D, :], in_=xhc[t % T_HC, 0])
            g_ps = lstm_step(t, wxhc_sb, shc_sb, hT[t % 2], hT[(t + 1) % 2])
        T_HC = repeat * T_HC

        # Q_0 from the final hindcast h, feeds forecast step 0
        emit_q(g_ps, hT[T_HC % 2], T_HC, None)

        # ---- autoregressive forecast ----
        for j in range(T_FC):
            t = T_HC + j
            xbuf = xaug[t % NXB]
            nc.sync.dma_start(out=xbuf[1 : 1 + D, :], in_=xfc[j, 0])
            nc.sync.dma_start(out=xbuf[KX + 1 : KX + 1 + D, :], in_=xfc[j, 1])
            nc.sync.dma_start(out=xbuf[65 : 65 + D, :], in_=xfc[j, 0])
            g_ps = lstm_step(t, wxfc_sb, sfc_sb, hT[t % 2], hT[(t + 1) % 2])
            emit_q(g_ps, hT[(t + 1) % 2], t + 1 if j + 1 < T_FC else None, j)

        nc.sync.dma_start(out=yout[:], in_=y_sb)

        pools.close()

    nc.finalize()
    return nc


def _prep_inputs(inputs):
    """Host-side weight folding + per-core input shards (bf16)."""
    f32 = np.float32
    x_d_hc = np.asarray(inputs["x_d_hc"], f32)
    x_d_fc = np.asarray(inputs["x_d_fc"], f32)
    x_s = np.asarray(inputs["x_s"], f32)
    W_emb_hc = np.asarray(inputs["W_emb_hc"], f32)
    b_emb_hc = np.asarray(inputs["b_emb_hc"], f32)
    W_emb_fc = np.asarray(inputs["W_emb_fc"], f32)
    b_emb_fc = np.asarray(inputs["b_emb_fc"], f32)
    W_emb_ar = np.asarray(inputs["W_emb_ar"], f32)
    b_emb_ar = np.asarray(inputs["b_emb_ar"], f32)
    W_ih = np.asarray(inputs["W_ih"], f32)
    W_hh = np.asarray(inputs["W_hh"], f32)
    b_ih = np.asarray(inputs["b_ih"], f32)
    b_hh = np.asarray(inputs["b_hh"], f32)
    W_out = np.asarray(inputs["W_out"], f32)
    b_out = np.asarray(inputs["b_out"], f32)

    W_E = W_ih[:, :E]
    W_S = W_ih[:, E:]
    v_q = 0.5 * (W_emb_ar @ W_E.T)  # [1, 4H], multiplies Q' + b_out
    # dynamic x weights: [Q | x_t] rows
    XW_hc = np.concatenate([np.zeros_like(v_q), W_emb_hc @ W_E.T], axis=0)  # [17, 4H]
    XW_fc = np.concatenate([v_q, 0.5 * (W_emb_fc @ W_E.T)], axis=0)

    # static gate contribution per batch row (exact, fp32):
    b_hc = b_emb_hc @ W_E.T + b_ih + b_hh
    b_fc = (
        0.5 * (b_emb_fc @ W_E.T)
        + 0.5 * (b_emb_ar @ W_E.T)
        + b_ih
        + b_hh
        + float(b_out[0]) * v_q[0]
    )
    S_hc = x_s @ W_S.T + b_hc[None]  # [B, 4H]
    S_fc = x_s @ W_S.T + b_fc[None]

    # gate permutation: reference order i,f,g,o -> kernel order g,f,i,o
    perm = np.concatenate(
        [np.arange(2 * H, 3 * H), np.arange(H, 2 * H), np.arange(0, H), np.arange(3 * H, 4 * H)]
    )
    XW_hc = XW_hc[:, perm]
    XW_fc = XW_fc[:, perm]
    S_hc = S_hc[:, perm]
    S_fc = S_fc[:, perm]
    WhhT = W_hh[perm].T  # [H, 4H] permuted on gate axis, natural on h axis
    whh_packed = np.concatenate([WhhT[0:128], WhhT[128:256]], axis=1)  # [128, 2*4H]
    wout_packed = np.stack([W_out[0:128, 0], W_out[128:256, 0]], axis=1)  # [128, 2]

    def hilo(a):
        hi = a.astype(F16)
        lo = (a - hi.astype(np.float32)).astype(F16)
        return np.stack([hi, lo], axis=0)

    def stack_pad(a):  # [17,4H] fp32 -> [81,4H] fp16 [hi; hi; 0-pad; lo]
        hl = hilo(a)
        out = np.zeros((64 + KX, a.shape[1]), F16)
        out[0:KX] = hl[0]
        out[KX : 2 * KX] = hl[0]
        out[64 : 64 + KX] = hl[1]
        return out

    wxhc_b = stack_pad(XW_hc)       # [81, 4H]
    wxfc_b = stack_pad(XW_fc)
    whh_b = hilo(whh_packed)        # [2, 128, 2*4H]
    wout_b = wout_packed.astype(F16)
    iden_b = np.eye(128, dtype=F16)

    def pack_static(S):
        # [BC, 4H] -> gate-major tile [128, 8*128]: [p, c*128+b] = S[b, c*128+p]
        St = S.T.reshape(NCH, 128, BC).transpose(1, 0, 2).reshape(128, NCH * BC)
        hi = St.astype(F16)
        lo = (St - hi.astype(np.float32)).astype(F16)
        return np.stack([hi, lo], axis=0)  # [2, 128, 1024]

    def pack_x(x):  # [BC, T, D] slice -> [T, 2, D, BC] hi/lo
        xt = np.ascontiguousarray(x.transpose(1, 2, 0)).astype(np.float32)  # [T, D, BC]
        hi = xt.astype(F16)
        lo = (xt - hi.astype(np.float32)).astype(F16)
        return np.stack([hi, lo], axis=1)  # [T, 2, D, BC]

    in_maps = []
    for core in range(NCORES):
        sl = slice(core * BC, (core + 1) * BC)
        xhc_t = pack_x(x_d_hc[sl])
        xfc_t = pack_x(x_d_fc[sl])
        in_maps.append(
            {
                "xhc": xhc_t,
                "xfc": xfc_t,
                "wxhc": wxhc_b,
                "wxfc": wxfc_b,
                "shc": pack_static(S_hc[sl]),
                "sfc": pack_static(S_fc[sl]),
                "iden": iden_b,
                "whh": whh_b,
                "wout": wout_b,
            }
        )
    return in_maps, float(b_out[0])


_CACHE = {}


def kernel(**inputs) -> np.ndarray:
    in_maps, b_out_val = _prep_inputs(inputs)
    if "nc" not in _CACHE:
        _CACHE["nc"] = _build()
    nc = _CACHE["nc"]
    res = run_bass_kernel_spmd(nc, in_maps, core_ids=list(range(NCORES)))
    outs = res.results
    # y_core: [48, 128] (time-major, without b_out) -> full [1024, 48, 1]
    y = np.empty((B, T_FC, 1), np.float32)
    for core in range(NCORES):
        y[core * BC : (core + 1) * BC, :, 0] = outs[core]["yout"].reshape(T_FC, BC).T
    y += b_out_val
    return y


if __name__ == "__main__":
    rng = np.random.default_rng(0)
    dummy = {
        "x_d_hc": rng.standard_normal((B, T_HC, D), np.float32),
        "x_d_fc": rng.standard_normal((B, T_FC, D), np.float32),
        "x_s": rng.standard_normal((B, S), np.float32),
        "W_emb_hc": rng.standard_normal((D, E), np.float32) * 0.25,
        "b_emb_hc": np.zeros(E, np.float32),
        "W_emb_fc": rng.standard_normal((D, E), np.float32) * 0.25,
        "b_emb_fc": np.zeros(E, np.float32),
        "W_emb_ar": rng.standard_normal((1, E), np.float32),
        "b_emb_ar": np.zeros(E, np.float32),
        "W_ih": rng.standard_normal((G4, E + S), np.float32) / 16,
        "W_hh": rng.standard_normal((G4, H), np.float32) / 16,
        "b_ih": np.zeros(G4, np.float32),
        "b_hh": np.zeros(G4, np.float32),
        "W_out": rng.standard_normal((H, 1), np.float32) / 16,
        "b_out": np.zeros(1, np.float32),
    }
    y = kernel(**dummy)
    print("kernel ran, y shape", y.shape, "mean", y.mean())

